# revision 13
# baseline (speedup 1.0000x reference)
"""ExpertGraphConv Trainium2 kernel.

Computation (per token n, experts E=16, D=512):
    adjacency = sigmoid(adj_logits)                       [E,E]
    a = x @ w1 ; c = x @ w2                               [N,E]
    gate[n,i,j] = adjacency[i,j]*sigmoid(a[n,i]+c[n,j]+b)*(1-eye)
    neighbor = einsum('nij,njd->nid', gate, x)
    out = gelu(neighbor @ Wn.T + x @ Ws.T + bn + bs)

Mapping (per core, data-parallel over tokens; core k takes batch k):
  rows = tokens*E = 8192 per core, processed in 64 blocks of 128 rows
  (8 tokens per block).  Per block:
    - PE-transpose x block to get xT (din on partitions)
    - joint a/c GEMM (lhsT = [w1|w2]) over 2-block superblocks
    - outer-sum a[i]+c[j]+b via a K=2 matmul, sigmoid via tanh (keeps
      the whole kernel in the single 'gelu_and_others' ACT table set)
    - gate = blockdiag(0.5*adjacency^T, zero diag) * (tanh+1)
    - xw = x @ Wn^T (psum->sbuf), h = x @ Ws^T + ones x bias + gate^T @ xw
    - out = gelu(h)
  Matmuls run as float32r (full-rate fp32 path on TRN2).
"""

import sys

sys.path.insert(0, "/opt/trn_rl_repo")

import numpy as np

import concourse.bacc as bacc
import concourse.mybir as mybir
import concourse.tile as tile
from concourse.masks import make_identity

F32 = mybir.dt.float32
F32R = mybir.dt.float32r

B, S, E, D = 8, 512, 16, 512
N_CORES = 8
ROWS_PER_CORE = (B // N_CORES) * S * E  # 8192
KC = D // 128  # 4 contraction chunks

AF = mybir.ActivationFunctionType


def build_program(n_rows=ROWS_PER_CORE, repeat=1, use_f32r=True,
                  final_act=None):
    """Build the per-core Bass program. Input x is the core's [n_rows, D]
    row-major shard; all small tensors are replicated."""
    assert n_rows % 256 == 0
    if final_act is None:
        final_act = AF.Gelu
    nc = bacc.Bacc("TRN2", target_bir_lowering=False, debug=False,
                   num_devices=N_CORES)

    x_d = nc.dram_tensor("x", [n_rows, D], F32, kind="ExternalInput").ap()
    wn_d = nc.dram_tensor("wn", [D, D], F32, kind="ExternalInput").ap()
    ws_d = nc.dram_tensor("ws", [D, D], F32, kind="ExternalInput").ap()
    mww_d = nc.dram_tensor("mww", [2 * D], F32, kind="ExternalInput").ap()
    bn_d = nc.dram_tensor("bn", [D], F32, kind="ExternalInput").ap()
    bs_d = nc.dram_tensor("bs", [D], F32, kind="ExternalInput").ap()
    mwb_d = nc.dram_tensor("mwb", [1, 1], F32, kind="ExternalInput").ap()
    adj_d = nc.dram_tensor("adj", [E, E], F32, kind="ExternalInput").ap()
    out_d = nc.dram_tensor("out", [n_rows, D], F32, kind="ExternalOutput").ap()

    MDT = F32R if use_f32r else F32

    with tile.TileContext(nc) as tc:
        from contextlib import ExitStack

        with ExitStack() as ctx:
            consts = ctx.enter_context(tc.tile_pool(name="consts", bufs=1))

            # ---- constants ----
            ident = consts.tile([128, 128], F32)
            make_identity(nc, ident)

            # weights natural [dout, din] -> [128, 4, D] (p=dout%128)
            wn_nat = consts.tile([128, KC, D], F32)
            ws_nat = consts.tile([128, KC, D], F32)
            nc.sync.dma_start(wn_nat[:], wn_d.rearrange("(o p) f -> p o f", p=128))
            nc.sync.dma_start(ws_nat[:], ws_d.rearrange("(o p) f -> p o f", p=128))

            # transposed weights W^T [din(p), chunk, dout]
            wnT = consts.tile([128, KC, D], MDT)
            wsT = consts.tile([128, KC, D], MDT)
            with tc.tile_pool(name="wps", bufs=2, space="PSUM") as wps:
                for nat, wT in ((wn_nat, wnT), (ws_nat, wsT)):
                    for k in range(KC):  # din chunk
                        ps = wps.tile([128, D], F32, tag="wps")
                        for j in range(KC):  # dout chunk
                            nc.tensor.transpose(
                                ps[:, j * 128:(j + 1) * 128],
                                nat[:, j, k * 128:(k + 1) * 128],
                                ident,
                            )
                        nc.vector.tensor_copy(wT[:, k, :], ps[:])

            # w12 [din(p), chunk, 2]
            w12f = consts.tile([128, KC, 2], F32)
            for c in range(2):
                nc.sync.dma_start(
                    w12f[:, :, c],
                    mww_d[c * D:(c + 1) * D].rearrange("(o p) -> p o", p=128))
            w12 = consts.tile([128, KC, 2], MDT)
            nc.vector.tensor_copy(w12[:], w12f[:])

            # bias row bn+bs [1, D]
            bias_row = consts.tile([1, D], MDT)
            btmp = consts.tile([1, D], F32)
            btmp2 = consts.tile([1, D], F32)
            nc.sync.dma_start(btmp[:], bn_d[None, :])
            nc.sync.dma_start(btmp2[:], bs_d[None, :])
            nc.vector.tensor_tensor(bias_row[:], btmp[:], btmp2[:],
                                    mybir.AluOpType.add)

            ones_f = consts.tile([1, 128], F32)
            nc.gpsimd.memset(ones_f[:], 1.0)
            ones_row = consts.tile([1, 128], MDT)
            nc.vector.tensor_copy(ones_row[:], ones_f[:])

            mwb = consts.tile([1, 1], F32)
            nc.sync.dma_start(mwb[:], mwb_d[:])
            # b/2 replicated to all partitions, for the per-partition tanh bias
            bhalf = consts.tile([128, 1], F32)
            nc.gpsimd.partition_broadcast(bhalf[:], mwb[:])
            nc.scalar.mul(bhalf[:], bhalf[:], 0.5)

            # A16[j,i] = 0.25*(tanh(adj_logits[i,j]/2)+1) = 0.5*sigmoid(adjL)^T,
            # diag zeroed.  Abd = 8x block-diagonal replication.
            adjT = consts.tile([E, E], F32)
            with nc.allow_non_contiguous_dma(reason="one-time 16x16 transpose load"):
                nc.sync.dma_start(adjT[:], adj_d.rearrange("i j -> j i"))
            a16 = consts.tile([E, E], F32)
            nc.scalar.activation(a16[:], adjT[:], AF.Tanh, scale=0.5)
            nc.vector.tensor_scalar(a16[:], a16[:], 1.0, 0.25,
                                    mybir.AluOpType.add, mybir.AluOpType.mult)
            nc.gpsimd.affine_select(
                out=a16, in_=a16, compare_op=mybir.AluOpType.not_equal,
                fill=0.0, base=0, pattern=[[-1, E]], channel_multiplier=1)
            abd = consts.tile([128, 128], F32)
            nc.gpsimd.memset(abd[:], 0.0)
            for t in range(8):
                nc.sync.dma_start(
                    abd[t * E:(t + 1) * E, t * E:(t + 1) * E], a16[:])

            # ---- main loop pools ----
            p_xn = ctx.enter_context(tc.tile_pool(name="p_xn", bufs=3))
            p_xt = ctx.enter_context(tc.tile_pool(name="p_xt", bufs=2))
            p_lr = ctx.enter_context(tc.tile_pool(name="p_lr", bufs=3))
            p_g = ctx.enter_context(tc.tile_pool(name="p_g", bufs=3))
            p_xw = ctx.enter_context(tc.tile_pool(name="p_xw", bufs=2))
            p_o = ctx.enter_context(tc.tile_pool(name="p_o", bufs=3))
            ps_t = ctx.enter_context(tc.tile_pool(name="ps_t", bufs=2, space="PSUM"))
            ps_ac = ctx.enter_context(tc.tile_pool(name="ps_ac", bufs=1, space="PSUM"))
            ps_z = ctx.enter_context(tc.tile_pool(name="ps_z", bufs=1, space="PSUM"))
            ps_xw = ctx.enter_context(tc.tile_pool(name="ps_xw", bufs=2, space="PSUM"))
            ps_h = ctx.enter_context(tc.tile_pool(name="ps_h", bufs=2, space="PSUM"))

            def body(_iv=None):
                for sb in range(n_rows // 256):
                    xt = p_xt.tile([128, KC, 256], MDT)
                    for b2 in range(2):
                        blk = sb * 2 + b2
                        xn = p_xn.tile([128, D], F32, tag="xn")
                        nc.sync.dma_start(
                            xn[:], x_d[blk * 128:(blk + 1) * 128, :])
                        pt = ps_t.tile([128, D], F32, tag="pt")
                        for k in range(KC):
                            nc.tensor.transpose(
                                pt[:, k * 128:(k + 1) * 128],
                                xn[:, k * 128:(k + 1) * 128], ident)
                        nc.vector.tensor_copy(
                            xt[:, :, b2 * 128:(b2 + 1) * 128],
                            pt.rearrange("p (o f) -> p o f", o=KC))

                    # a GEMM over both blocks: [1, 256] row form
                    pac = ps_ac.tile([1, 256], F32, tag="pac")
                    for k in range(KC):
                        nc.tensor.matmul(
                            pac[:], w12[:, k, 0:1], xt[:, k, :],
                            start=(k == 0), stop=(k == KC - 1))
                    a_row = p_lr.tile([1, 256], F32, tag="a_row")
                    nc.scalar.copy(a_row[:], pac[:])

                    for b2 in range(2):
                        blk = sb * 2 + b2
                        bsl = slice(b2 * 128, (b2 + 1) * 128)

                        # xw = x @ Wn^T
                        pxw = ps_xw.tile([128, D], F32, tag="pxw")
                        for k in range(KC):
                            nc.tensor.matmul(
                                pxw[:], xt[:, k, bsl], wnT[:, k, :],
                                start=(k == 0), stop=(k == KC - 1))

                        # a/c columns [128, 2] (c in column 1; fp32r moving
                        # free dim must be >= 2)
                        pcc = ps_z.tile([128, 2], F32, tag="pcc")
                        for k in range(KC):
                            nc.tensor.matmul(
                                pcc[:], xt[:, k, bsl], w12[:, k, :],
                                start=(k == 0), stop=(k == KC - 1))
                        cb = p_lr.tile([128, 1], F32, tag="cb")
                        nc.vector.tensor_scalar(cb[:], pcc[:, 1:2], 0.5,
                                                bhalf[:],
                                                mybir.AluOpType.mult,
                                                mybir.AluOpType.add)

                        # t[j,i] = tanh((a[i] + c[j] + b)/2);  a broadcast
                        # along partitions, c+b as per-partition ACT bias
                        a_bc = p_g.tile([128, 128], F32, tag="a_bc")
                        nc.gpsimd.partition_broadcast(a_bc[:], a_row[0:1, bsl])
                        tt = p_g.tile([128, 128], F32, tag="tt")
                        gate = p_g.tile([128, 128], MDT, tag="gate")
                        nc.scalar.activation(tt[:], a_bc[:], AF.Tanh,
                                             scale=0.5, bias=cb[:])
                        nc.vector.tensor_scalar(gate[:], tt[:], 1.0, None,
                                                mybir.AluOpType.add)
                        nc.vector.tensor_tensor(gate[:], gate[:], abd[:],
                                                mybir.AluOpType.mult)

                        xw = p_xw.tile([128, D], MDT, tag="xw")
                        nc.vector.tensor_copy(xw[:], pxw[:])

                        # h = x@Ws^T + ones x bias + gate^T @ xw
                        ph = ps_h.tile([128, D], F32, tag="ph")
                        for k in range(KC):
                            nc.tensor.matmul(
                                ph[:], xt[:, k, bsl], wsT[:, k, :],
                                start=(k == 0), stop=False)
                        nc.tensor.matmul(ph[:], ones_row[:],
                                         bias_row[:],
                                         start=False, stop=False)
                        nc.tensor.matmul(ph[:], gate[:], xw[:],
                                         start=False, stop=True)

                        ot = p_o.tile([128, D], F32, tag="ot")
                        nc.scalar.activation(ot[:], ph[:], final_act)
                        nc.sync.dma_start(
                            out_d[blk * 128:(blk + 1) * 128, :], ot[:])

            if repeat == 1:
                body()
            else:
                with tc.For_i(0, repeat, 1):
                    body()

    nc.compile()
    return nc


_PROGRAMS = {}


def _get_program(n_rows=ROWS_PER_CORE, repeat=1, use_f32r=True):
    key = (n_rows, repeat, use_f32r)
    if key not in _PROGRAMS:
        _PROGRAMS[key] = build_program(n_rows, repeat, use_f32r)
    return _PROGRAMS[key]


def make_in_maps(expert_features, Wn, bn, Ws, bs, mw_w, mw_b, adj_logits,
                 n_cores=N_CORES):
    x = np.ascontiguousarray(np.asarray(expert_features, dtype=np.float32))
    x = x.reshape(B * S * E, D)
    rows = x.shape[0] // n_cores
    common = {
        "wn": np.ascontiguousarray(np.asarray(Wn, dtype=np.float32)),
        "ws": np.ascontiguousarray(np.asarray(Ws, dtype=np.float32)),
        "mww": np.ascontiguousarray(np.asarray(mw_w, dtype=np.float32)),
        "bn": np.ascontiguousarray(np.asarray(bn, dtype=np.float32)),
        "bs": np.ascontiguousarray(np.asarray(bs, dtype=np.float32)),
        "mwb": np.asarray(mw_b, dtype=np.float32).reshape(1, 1),
        "adj": np.ascontiguousarray(np.asarray(adj_logits, dtype=np.float32)),
    }
    return [
        {"x": np.ascontiguousarray(x[k * rows:(k + 1) * rows]), **common}
        for k in range(n_cores)
    ]


def kernel(expert_features, Wn, bn, Ws, bs, mw_w, mw_b, adj_logits):
    from concourse.bass_utils import run_bass_kernel_spmd

    nc = _get_program()
    in_maps = make_in_maps(expert_features, Wn, bn, Ws, bs, mw_w, mw_b,
                           adj_logits)
    res = run_bass_kernel_spmd(nc, in_maps, core_ids=list(range(N_CORES)))
    outs = [r["out"].reshape(B // N_CORES, S, E, D) for r in res.results]
    return np.concatenate(outs, axis=0)
